# revision 28
# baseline (speedup 1.0000x reference)
"""EventSequenceEmbedder Trainium2 kernel (8-core data-parallel).

Strategy
--------
The reference computes, per (batch, event, card):
    h = concat(card_emb, hero_emb, acting_emb, nump_emb, scalars@Ws,
               bets@Wb, action@Wa) @ W_combine + b_combine
    out = LayerNorm(h) + source_emb,  zeroed for padded events.

Everything feeding h is linear, so the whole pre-LN graph folds into ONE
small matmul per output row:
    h[r, :] = A[r, :] @ W_tilde            A[r] in R^120
where A[r] packs one-hot card/hero/acting/nump ids, a bias-1, and the 19
raw float features (split into bf16 hi+lo rows for fp32-level accuracy),
and W_tilde rows are the host-projected tables (card_table@Wc, etc.).
W_tilde rows are mean-centered on the host so mean(h) == 0 analytically,
and W_tilde is split hi/lo into two bf16 matmuls accumulating in fp32
PSUM.  Masked (padded) events get all-zero A columns -> h == 0 -> LN
output 0.

Per 128-row tile on device (tiles processed in groups of ~7 that share
one lhs-load DMA, one store DMA pair, and batched sqrt/reciprocal):
    PE:  2 bf16 matmuls (W_hi half, W_lo half) -> h in PSUM
    ACT: Square with accum_out -> sum(h^2) per row (one tile per big
         group uses DVE bn_stats instead, to balance engine load);
         batched Sqrt(ssq/256 + eps)
    DVE: batched reciprocal -> rstd; srcm = src * mask (bf16);
         fused scalar_tensor_tensor: out = (h * rstd) + srcm
    DMA: grouped HWDGE loads/stores sized to amortize per-DMA overhead

Sharding: batch dim 32 -> 4 batches per core x 8 cores (data parallel,
weights replicated).
"""

import numpy as np
from ml_dtypes import bfloat16

import concourse.bass as bass
import concourse.tile as tile
from concourse import bacc, mybir
from concourse.bass_utils import run_bass_kernel_spmd

B, E, C, D, NA, MP = 32, 256, 7, 256, 8, 9
NCORES = 8
BC = B // NCORES                 # batches per core
ROWS = BC * E * C                # 7168 output rows per core
NTILES = ROWS // 128             # 56
K = 120                          # lhsT contraction dim
LN_EPS = 1e-5

_kernel_cache = {}


def _build_bass():
    nc = bacc.Bacc(None, target_bir_lowering=False, debug=False)
    f32 = mybir.dt.float32
    bf16 = mybir.dt.bfloat16

    lhsT = nc.declare_dram_parameter("lhsT", [K, ROWS], bf16, isOutput=False)
    w_hl = nc.declare_dram_parameter("w_hl", [K, 2 * D], bf16, isOutput=False)
    src_ext = nc.declare_dram_parameter("src_ext", [134, D], bf16, isOutput=False)
    m_cols = nc.declare_dram_parameter("m_cols", [128, NTILES], f32, isOutput=False)

    out = nc.declare_dram_parameter("out", [ROWS, D], f32, isOutput=True)

    G = 7                                    # tiles per group (DMA + stats batch)
    NG = NTILES // G                         # 8 groups
    with tile.TileContext(nc) as tc:
        with (
            tc.tile_pool(name="consts", bufs=1) as consts,
            tc.tile_pool(name="lhs", bufs=4) as lhs_pool,
            tc.tile_pool(name="psum", bufs=8, space="PSUM") as psum_pool,
            tc.tile_pool(name="stats", bufs=4) as stats_pool,
            tc.tile_pool(name="outs", bufs=3) as out_pool,
            tc.tile_pool(name="srcm_p", bufs=14) as srcm_pool,
            tc.tile_pool(name="scratch", bufs=1) as scratch_pool,
        ):
            # w + src + mcol first: they unblock DVE's srcm prefetch work;
            # lhs tiles follow (first matmul needs ~3us of DMA anyway)
            # source-row variants, packed so block w starts at src_ext row w.
            # Tile t (t % 7 == j) needs rows (128t+p) % 7 = (2j + p) % 7,
            # i.e. block w = (2j) % 7.
            src_sb = consts.tile([128, 7, D], bf16, tag="src")
            src_base = src_ext[:, :]
            nc.sync.dma_start(
                out=src_sb,
                in_=bass.AP(
                    tensor=src_base.tensor, offset=src_base.offset,
                    ap=[[D, 128], [D, 7], [1, D]],
                ),
            )
            whl_sb = consts.tile([K, 2 * D], bf16, tag="whl")
            nc.sync.dma_start(out=whl_sb, in_=w_hl[:, :])
            mcol_sb = consts.tile([128, NTILES], f32, tag="mcol")
            nc.sync.dma_start(out=mcol_sb, in_=m_cols[:, :])
            eps_sb = consts.tile([128, 1], f32, tag="eps")
            nc.vector.memset(eps_sb, LN_EPS)
            lhs_tiles = {}
            SIZES = [2, 5, 7, 7, 7, 7, 7, 7, 5, 2]
            OFFS = [sum(SIZES[:i]) for i in range(len(SIZES))]
            NGV = len(SIZES)
            for gp in range(3):
                t0, sz = OFFS[gp], SIZES[gp]
                lhs_sb = lhs_pool.tile([K, sz * 128], bf16, tag="lhs")
                nc.sync.dma_start(
                    out=lhs_sb, in_=lhsT[:, t0 * 128:(t0 + sz) * 128]
                )
                lhs_tiles[gp] = lhs_sb
            sq_garbage = scratch_pool.tile([128, D], bf16, tag="sqg")

            for g in range(NGV):
                t0, sz = OFFS[g], SIZES[g]
                lhs_sb = lhs_tiles.pop(g)
                ssq = stats_pool.tile([128, sz], f32, tag="ssq")
                rstd = stats_pool.tile([128, sz], f32, tag="rstd")
                out_g = out_pool.tile([128, sz, D], f32, tag="out")
                # srcm tiles depend only on constants — emit first so the
                # vector engine can run them while PE/ACT fill the group
                srcms = []
                for j in range(sz):
                    t = t0 + j
                    srcm = srcm_pool.tile([128, D], bf16, tag="srcm")
                    nc.vector.tensor_scalar_mul(
                        out=srcm, in0=src_sb[:, (2 * t) % 7, :],
                        scalar1=mcol_sb[:, t:t + 1],
                    )
                    srcms.append(srcm)
                h_tiles = []
                for j in range(sz):
                    h_ps = psum_pool.tile([128, D], f32, tag="h")
                    h_tiles.append(h_ps)
                nb1 = min(3, sz)          # first sqrt/recip batch size
                for j in range(sz):
                    lhsT_t = lhs_sb[:, j * 128:(j + 1) * 128]
                    h_ps = h_tiles[j]
                    nc.tensor.matmul(h_ps, lhsT_t, whl_sb[:, 0:D],
                                     start=True, stop=False)
                    nc.tensor.matmul(h_ps, lhsT_t, whl_sb[:, D:2 * D],
                                     start=False, stop=True)
                    # sum of squares per row (mean is 0 by weight centering);
                    # the last tile of big groups goes through DVE bn_stats
                    # to offload the ACT engine
                    if j < sz - 1 or sz < 7:
                        nc.scalar.activation(
                            out=sq_garbage, in_=h_ps,
                            func=mybir.ActivationFunctionType.Square,
                            accum_out=ssq[:, j:j + 1],
                        )
                    else:
                        st6 = stats_pool.tile([128, 6], f32, tag="st6")
                        nc.vector.bn_stats(out=st6, in_=h_ps)
                        mv = stats_pool.tile([128, 2], f32, tag="mv")
                        nc.vector.bn_aggr(out=mv, in_=st6)
                        nc.vector.tensor_scalar_mul(
                            out=ssq[:, j:j + 1], in0=mv[:, 1:2],
                            scalar1=float(D),
                        )
                    if j == nb1 - 1:
                        # early rstd for the first batch: shortens the
                        # stats -> STT dependency chain
                        nc.scalar.activation(
                            out=rstd[:, 0:nb1], in_=ssq[:, 0:nb1],
                            func=mybir.ActivationFunctionType.Sqrt,
                            bias=eps_sb[:, :], scale=1.0 / D,
                        )
                        nc.vector.reciprocal(out=rstd[:, 0:nb1],
                                             in_=rstd[:, 0:nb1])
                if sz > nb1:
                    nc.scalar.activation(
                        out=rstd[:, nb1:sz], in_=ssq[:, nb1:sz],
                        func=mybir.ActivationFunctionType.Sqrt,
                        bias=eps_sb[:, :], scale=1.0 / D,
                    )
                    nc.vector.reciprocal(out=rstd[:, nb1:sz],
                                         in_=rstd[:, nb1:sz])
                for j in range(sz):
                    nc.vector.scalar_tensor_tensor(
                        out=out_g[:, j, :], in0=h_tiles[j],
                        scalar=rstd[:, j:j + 1], in1=srcms[j],
                        op0=mybir.AluOpType.mult, op1=mybir.AluOpType.add,
                    )
                base = t0 * 128
                nc.sync.dma_start(
                    out=out[base:base + nb1 * 128, :].rearrange(
                        "(j p) d -> p j d", j=nb1),
                    in_=out_g[:, 0:nb1, :],
                )
                if sz > nb1:
                    nc.sync.dma_start(
                        out=out[base + nb1 * 128:base + sz * 128, :].rearrange(
                            "(j p) d -> p j d", j=sz - nb1),
                        in_=out_g[:, nb1:sz, :],
                    )
                if g + 3 < NGV:
                    tn, szn = OFFS[g + 3], SIZES[g + 3]
                    lhs_nx = lhs_pool.tile([K, szn * 128], bf16, tag="lhs")
                    nc.sync.dma_start(
                        out=lhs_nx, in_=lhsT[:, tn * 128:(tn + szn) * 128],
                    )
                    lhs_tiles[g + 3] = lhs_nx
    nc.finalize()
    return nc


def _prepare(inputs):
    """Host-side weight folding + per-core input shards."""
    f32 = np.float32
    W = np.asarray(inputs["W_combine"], f32)
    Wc = W[0:256]
    Wx = [W[256 * (i + 1):256 * (i + 2)] for i in range(6)]

    R = np.zeros((101, D), f32)
    R[0:53] = np.asarray(inputs["card_table"], f32) @ Wc
    R[53:62] = np.asarray(inputs["hero_table"], f32) @ Wx[0]
    R[62:71] = np.asarray(inputs["acting_table"], f32) @ Wx[1]
    R[71:81] = np.asarray(inputs["nump_table"], f32) @ Wx[2]
    R[81] = (np.asarray(inputs["b_scalar"], f32) @ Wx[3]
             + np.asarray(inputs["b_bet"], f32) @ Wx[4]
             + np.asarray(inputs["b_action"], f32) @ Wx[5]
             + np.asarray(inputs["b_combine"], f32))
    R[82:84] = np.asarray(inputs["W_scalar"], f32) @ Wx[3]
    R[84:93] = np.asarray(inputs["W_bet"], f32) @ Wx[4]
    R[93:101] = np.asarray(inputs["W_action"], f32) @ Wx[5]
    R = R - R.mean(axis=1, keepdims=True)

    rhs_full = np.vstack([R, R[82:101]])            # (120, 256)
    w_hi = rhs_full.astype(bfloat16)
    w_lo = (rhs_full - w_hi.astype(f32)).astype(bfloat16)
    w_hl = np.concatenate([w_hi, w_lo], axis=1)     # (120, 512)

    # source rows extended periodically so every tile variant is a slice
    src_ids = np.array([0, 0, 0, 0, 0, 1, 1])
    src7 = np.asarray(inputs["source_table"], f32)[src_ids]      # (7, 256)
    src_ext = src7[np.arange(134) % 7].astype(bfloat16)          # (134, 256)

    emask = (np.arange(E)[None, :]
             < np.asarray(inputs["seq_lengths"])[:, None]).astype(f32)  # (B, E)

    card = np.asarray(inputs["card_ids"]).reshape(B, E * C)
    hero = np.repeat(np.asarray(inputs["hero_pos"]), C, axis=1)  # (B, E*C)
    act = np.repeat(np.asarray(inputs["acting_pos"]), C, axis=1)
    nump = np.repeat(np.asarray(inputs["num_players"]), C, axis=1)
    feats = np.concatenate(
        [np.asarray(inputs["scalars"], f32),
         np.asarray(inputs["bets"], f32),
         np.asarray(inputs["action"], f32)], axis=-1)            # (B, E, 19)
    f_hi = feats.astype(bfloat16).astype(f32)
    f_lo = (feats - f_hi)

    in_maps = []
    for m in range(NCORES):
        bs = slice(m * BC, (m + 1) * BC)
        n = ROWS
        cols = np.arange(n)
        lhsT = np.zeros((K, n), f32)
        lhsT[card[bs].reshape(-1), cols] = 1.0
        lhsT[53 + hero[bs].reshape(-1), cols] = 1.0
        lhsT[62 + act[bs].reshape(-1), cols] = 1.0
        lhsT[71 + nump[bs].reshape(-1), cols] = 1.0
        lhsT[81, :] = 1.0
        lhsT[82:101, :] = np.repeat(
            f_hi[bs].reshape(-1, 19), C, axis=0).T
        lhsT[101:120, :] = np.repeat(
            f_lo[bs].reshape(-1, 19), C, axis=0).T
        mrow = np.repeat(emask[bs].reshape(-1), C)               # (7168,)
        lhsT *= mrow[None, :]
        m_cols = mrow.reshape(NTILES, 128).T.copy()              # (128, 56)
        in_maps.append({
            "lhsT": lhsT.astype(bfloat16),
            "w_hl": w_hl,
            "src_ext": src_ext,
            "m_cols": m_cols.astype(f32),
        })
    mask_full = np.repeat(emask, C, axis=1).astype(f32)          # (B, E*C)
    return in_maps, mask_full


def kernel(**inputs):
    if "nc" not in _kernel_cache:
        _kernel_cache["nc"] = _build_bass()
    nc = _kernel_cache["nc"]
    in_maps, mask = _prepare(inputs)
    res = run_bass_kernel_spmd(nc, in_maps, list(range(NCORES)))
    embs = np.concatenate(
        [res.results[m]["out"].reshape(BC, E * C, D) for m in range(NCORES)],
        axis=0)
    return embs, mask


# revision 29
# speedup vs baseline: 1.0083x; 1.0083x over previous
"""EventSequenceEmbedder Trainium2 kernel (8-core data-parallel).

Strategy
--------
The reference computes, per (batch, event, card):
    h = concat(card_emb, hero_emb, acting_emb, nump_emb, scalars@Ws,
               bets@Wb, action@Wa) @ W_combine + b_combine
    out = LayerNorm(h) + source_emb,  zeroed for padded events.

Everything feeding h is linear, so the whole pre-LN graph folds into ONE
small matmul per output row:
    h[r, :] = A[r, :] @ W_tilde            A[r] in R^120
where A[r] packs one-hot card/hero/acting/nump ids, a bias-1, and the 19
raw float features (split into bf16 hi+lo rows for fp32-level accuracy),
and W_tilde rows are the host-projected tables (card_table@Wc, etc.).
W_tilde rows are mean-centered on the host so mean(h) == 0 analytically,
and W_tilde is split hi/lo into two bf16 matmuls accumulating in fp32
PSUM.  Masked (padded) events get all-zero A columns -> h == 0 -> LN
output 0.

Per 128-row tile on device (tiles processed in groups of ~7 that share
one lhs-load DMA, one store DMA pair, and batched sqrt/reciprocal):
    PE:  2 bf16 matmuls (W_hi half, W_lo half) -> h in PSUM
    ACT: Square with accum_out -> sum(h^2) per row (one tile per big
         group uses DVE bn_stats instead, to balance engine load);
         batched Sqrt(ssq/256 + eps)
    DVE: batched reciprocal -> rstd; srcm = src * mask (bf16);
         fused scalar_tensor_tensor: out = (h * rstd) + srcm
    DMA: grouped HWDGE loads/stores sized to amortize per-DMA overhead

Sharding: batch dim 32 -> 4 batches per core x 8 cores (data parallel,
weights replicated).
"""

import numpy as np
from ml_dtypes import bfloat16

import concourse.bass as bass
import concourse.tile as tile
from concourse import bacc, mybir
from concourse.bass_utils import run_bass_kernel_spmd

B, E, C, D, NA, MP = 32, 256, 7, 256, 8, 9
NCORES = 8
BC = B // NCORES                 # batches per core
ROWS = BC * E * C                # 7168 output rows per core
NTILES = ROWS // 128             # 56
K = 120                          # lhsT contraction dim
LN_EPS = 1e-5

_kernel_cache = {}


def _build_bass():
    nc = bacc.Bacc(None, target_bir_lowering=False, debug=False)
    f32 = mybir.dt.float32
    bf16 = mybir.dt.bfloat16

    lhsT = nc.declare_dram_parameter("lhsT", [K, ROWS], bf16, isOutput=False)
    w_hl = nc.declare_dram_parameter("w_hl", [K, 2 * D], bf16, isOutput=False)
    src_ext = nc.declare_dram_parameter("src_ext", [134, D], bf16, isOutput=False)
    m_cols = nc.declare_dram_parameter("m_cols", [128, NTILES], f32, isOutput=False)

    out = nc.declare_dram_parameter("out", [ROWS, D], f32, isOutput=True)

    G = 7                                    # tiles per group (DMA + stats batch)
    NG = NTILES // G                         # 8 groups
    with tile.TileContext(nc) as tc:
        with (
            tc.tile_pool(name="consts", bufs=1) as consts,
            tc.tile_pool(name="lhs", bufs=4) as lhs_pool,
            tc.tile_pool(name="psum", bufs=8, space="PSUM") as psum_pool,
            tc.tile_pool(name="stats", bufs=4) as stats_pool,
            tc.tile_pool(name="outs", bufs=3) as out_pool,
            tc.tile_pool(name="srcm_p", bufs=14) as srcm_pool,
            tc.tile_pool(name="scratch", bufs=1) as scratch_pool,
        ):
            # w + src + mcol first: they unblock DVE's srcm prefetch work;
            # lhs tiles follow (first matmul needs ~3us of DMA anyway)
            # source-row variants, packed so block w starts at src_ext row w.
            # Tile t (t % 7 == j) needs rows (128t+p) % 7 = (2j + p) % 7,
            # i.e. block w = (2j) % 7.
            src_sb = consts.tile([128, 7, D], bf16, tag="src")
            src_base = src_ext[:, :]
            nc.sync.dma_start(
                out=src_sb,
                in_=bass.AP(
                    tensor=src_base.tensor, offset=src_base.offset,
                    ap=[[D, 128], [D, 7], [1, D]],
                ),
            )
            whl_sb = consts.tile([K, 2 * D], bf16, tag="whl")
            nc.sync.dma_start(out=whl_sb, in_=w_hl[:, :])
            mcol_sb = consts.tile([128, NTILES], f32, tag="mcol")
            nc.sync.dma_start(out=mcol_sb, in_=m_cols[:, :])
            eps_sb = consts.tile([128, 1], f32, tag="eps")
            nc.vector.memset(eps_sb, LN_EPS)
            lhs_tiles = {}
            SIZES = [1, 2, 4, 7, 7, 7, 7, 7, 7, 5, 2]
            OFFS = [sum(SIZES[:i]) for i in range(len(SIZES))]
            NGV = len(SIZES)
            for gp in range(3):
                t0, sz = OFFS[gp], SIZES[gp]
                lhs_sb = lhs_pool.tile([K, sz * 128], bf16, tag="lhs")
                nc.sync.dma_start(
                    out=lhs_sb, in_=lhsT[:, t0 * 128:(t0 + sz) * 128]
                )
                lhs_tiles[gp] = lhs_sb
            sq_garbage = scratch_pool.tile([128, D], bf16, tag="sqg")

            for g in range(NGV):
                t0, sz = OFFS[g], SIZES[g]
                lhs_sb = lhs_tiles.pop(g)
                ssq = stats_pool.tile([128, sz], f32, tag="ssq")
                rstd = stats_pool.tile([128, sz], f32, tag="rstd")
                out_g = out_pool.tile([128, sz, D], f32, tag="out")
                # srcm tiles depend only on constants — emit first so the
                # vector engine can run them while PE/ACT fill the group
                srcms = []
                for j in range(sz):
                    t = t0 + j
                    srcm = srcm_pool.tile([128, D], bf16, tag="srcm")
                    nc.vector.tensor_scalar_mul(
                        out=srcm, in0=src_sb[:, (2 * t) % 7, :],
                        scalar1=mcol_sb[:, t:t + 1],
                    )
                    srcms.append(srcm)
                h_tiles = []
                for j in range(sz):
                    h_ps = psum_pool.tile([128, D], f32, tag="h")
                    h_tiles.append(h_ps)
                nb1 = min(3, sz)          # first sqrt/recip batch size
                for j in range(sz):
                    lhsT_t = lhs_sb[:, j * 128:(j + 1) * 128]
                    h_ps = h_tiles[j]
                    nc.tensor.matmul(h_ps, lhsT_t, whl_sb[:, 0:D],
                                     start=True, stop=False)
                    nc.tensor.matmul(h_ps, lhsT_t, whl_sb[:, D:2 * D],
                                     start=False, stop=True)
                    # sum of squares per row (mean is 0 by weight centering);
                    # the last tile of big groups goes through DVE bn_stats
                    # to offload the ACT engine
                    if j < sz - 1 or sz < 7:
                        nc.scalar.activation(
                            out=sq_garbage, in_=h_ps,
                            func=mybir.ActivationFunctionType.Square,
                            accum_out=ssq[:, j:j + 1],
                        )
                    else:
                        st6 = stats_pool.tile([128, 6], f32, tag="st6")
                        nc.vector.bn_stats(out=st6, in_=h_ps)
                        mv = stats_pool.tile([128, 2], f32, tag="mv")
                        nc.vector.bn_aggr(out=mv, in_=st6)
                        nc.vector.tensor_scalar_mul(
                            out=ssq[:, j:j + 1], in0=mv[:, 1:2],
                            scalar1=float(D),
                        )
                    if j == nb1 - 1:
                        # early rstd for the first batch: shortens the
                        # stats -> STT dependency chain
                        nc.scalar.activation(
                            out=rstd[:, 0:nb1], in_=ssq[:, 0:nb1],
                            func=mybir.ActivationFunctionType.Sqrt,
                            bias=eps_sb[:, :], scale=1.0 / D,
                        )
                        nc.vector.reciprocal(out=rstd[:, 0:nb1],
                                             in_=rstd[:, 0:nb1])
                if sz > nb1:
                    nc.scalar.activation(
                        out=rstd[:, nb1:sz], in_=ssq[:, nb1:sz],
                        func=mybir.ActivationFunctionType.Sqrt,
                        bias=eps_sb[:, :], scale=1.0 / D,
                    )
                    nc.vector.reciprocal(out=rstd[:, nb1:sz],
                                         in_=rstd[:, nb1:sz])
                for j in range(sz):
                    nc.vector.scalar_tensor_tensor(
                        out=out_g[:, j, :], in0=h_tiles[j],
                        scalar=rstd[:, j:j + 1], in1=srcms[j],
                        op0=mybir.AluOpType.mult, op1=mybir.AluOpType.add,
                    )
                base = t0 * 128
                nc.sync.dma_start(
                    out=out[base:base + nb1 * 128, :].rearrange(
                        "(j p) d -> p j d", j=nb1),
                    in_=out_g[:, 0:nb1, :],
                )
                if sz > nb1:
                    nc.sync.dma_start(
                        out=out[base + nb1 * 128:base + sz * 128, :].rearrange(
                            "(j p) d -> p j d", j=sz - nb1),
                        in_=out_g[:, nb1:sz, :],
                    )
                if g + 3 < NGV:
                    tn, szn = OFFS[g + 3], SIZES[g + 3]
                    lhs_nx = lhs_pool.tile([K, szn * 128], bf16, tag="lhs")
                    nc.sync.dma_start(
                        out=lhs_nx, in_=lhsT[:, tn * 128:(tn + szn) * 128],
                    )
                    lhs_tiles[g + 3] = lhs_nx
    nc.finalize()
    return nc


def _prepare(inputs):
    """Host-side weight folding + per-core input shards."""
    f32 = np.float32
    W = np.asarray(inputs["W_combine"], f32)
    Wc = W[0:256]
    Wx = [W[256 * (i + 1):256 * (i + 2)] for i in range(6)]

    R = np.zeros((101, D), f32)
    R[0:53] = np.asarray(inputs["card_table"], f32) @ Wc
    R[53:62] = np.asarray(inputs["hero_table"], f32) @ Wx[0]
    R[62:71] = np.asarray(inputs["acting_table"], f32) @ Wx[1]
    R[71:81] = np.asarray(inputs["nump_table"], f32) @ Wx[2]
    R[81] = (np.asarray(inputs["b_scalar"], f32) @ Wx[3]
             + np.asarray(inputs["b_bet"], f32) @ Wx[4]
             + np.asarray(inputs["b_action"], f32) @ Wx[5]
             + np.asarray(inputs["b_combine"], f32))
    R[82:84] = np.asarray(inputs["W_scalar"], f32) @ Wx[3]
    R[84:93] = np.asarray(inputs["W_bet"], f32) @ Wx[4]
    R[93:101] = np.asarray(inputs["W_action"], f32) @ Wx[5]
    R = R - R.mean(axis=1, keepdims=True)

    rhs_full = np.vstack([R, R[82:101]])            # (120, 256)
    w_hi = rhs_full.astype(bfloat16)
    w_lo = (rhs_full - w_hi.astype(f32)).astype(bfloat16)
    w_hl = np.concatenate([w_hi, w_lo], axis=1)     # (120, 512)

    # source rows extended periodically so every tile variant is a slice
    src_ids = np.array([0, 0, 0, 0, 0, 1, 1])
    src7 = np.asarray(inputs["source_table"], f32)[src_ids]      # (7, 256)
    src_ext = src7[np.arange(134) % 7].astype(bfloat16)          # (134, 256)

    emask = (np.arange(E)[None, :]
             < np.asarray(inputs["seq_lengths"])[:, None]).astype(f32)  # (B, E)

    card = np.asarray(inputs["card_ids"]).reshape(B, E * C)
    hero = np.repeat(np.asarray(inputs["hero_pos"]), C, axis=1)  # (B, E*C)
    act = np.repeat(np.asarray(inputs["acting_pos"]), C, axis=1)
    nump = np.repeat(np.asarray(inputs["num_players"]), C, axis=1)
    feats = np.concatenate(
        [np.asarray(inputs["scalars"], f32),
         np.asarray(inputs["bets"], f32),
         np.asarray(inputs["action"], f32)], axis=-1)            # (B, E, 19)
    f_hi = feats.astype(bfloat16).astype(f32)
    f_lo = (feats - f_hi)

    in_maps = []
    for m in range(NCORES):
        bs = slice(m * BC, (m + 1) * BC)
        n = ROWS
        cols = np.arange(n)
        lhsT = np.zeros((K, n), f32)
        lhsT[card[bs].reshape(-1), cols] = 1.0
        lhsT[53 + hero[bs].reshape(-1), cols] = 1.0
        lhsT[62 + act[bs].reshape(-1), cols] = 1.0
        lhsT[71 + nump[bs].reshape(-1), cols] = 1.0
        lhsT[81, :] = 1.0
        lhsT[82:101, :] = np.repeat(
            f_hi[bs].reshape(-1, 19), C, axis=0).T
        lhsT[101:120, :] = np.repeat(
            f_lo[bs].reshape(-1, 19), C, axis=0).T
        mrow = np.repeat(emask[bs].reshape(-1), C)               # (7168,)
        lhsT *= mrow[None, :]
        m_cols = mrow.reshape(NTILES, 128).T.copy()              # (128, 56)
        in_maps.append({
            "lhsT": lhsT.astype(bfloat16),
            "w_hl": w_hl,
            "src_ext": src_ext,
            "m_cols": m_cols.astype(f32),
        })
    mask_full = np.repeat(emask, C, axis=1).astype(f32)          # (B, E*C)
    return in_maps, mask_full


def kernel(**inputs):
    if "nc" not in _kernel_cache:
        _kernel_cache["nc"] = _build_bass()
    nc = _kernel_cache["nc"]
    in_maps, mask = _prepare(inputs)
    res = run_bass_kernel_spmd(nc, in_maps, list(range(NCORES)))
    embs = np.concatenate(
        [res.results[m]["out"].reshape(BC, E * C, D) for m in range(NCORES)],
        axis=0)
    return embs, mask


# revision 30
# speedup vs baseline: 1.0176x; 1.0092x over previous
"""EventSequenceEmbedder Trainium2 kernel (8-core data-parallel).

Strategy
--------
The reference computes, per (batch, event, card):
    h = concat(card_emb, hero_emb, acting_emb, nump_emb, scalars@Ws,
               bets@Wb, action@Wa) @ W_combine + b_combine
    out = LayerNorm(h) + source_emb,  zeroed for padded events.

Everything feeding h is linear, so the whole pre-LN graph folds into ONE
small matmul per output row:
    h[r, :] = A[r, :] @ W_tilde            A[r] in R^120
where A[r] packs one-hot card/hero/acting/nump ids, a bias-1, and the 19
raw float features (split into bf16 hi+lo rows for fp32-level accuracy),
and W_tilde rows are the host-projected tables (card_table@Wc, etc.).
W_tilde rows are mean-centered on the host so mean(h) == 0 analytically,
and W_tilde is split hi/lo into two bf16 matmuls accumulating in fp32
PSUM.  Masked (padded) events get all-zero A columns -> h == 0 -> LN
output 0.

Per 128-row tile on device (tiles processed in groups of ~7 that share
one lhs-load DMA, one store DMA pair, and batched sqrt/reciprocal):
    PE:  2 bf16 matmuls (W_hi half, W_lo half) -> h in PSUM
    ACT: Square with accum_out -> sum(h^2) per row (one tile per big
         group uses DVE bn_stats instead, to balance engine load);
         batched Sqrt(ssq/256 + eps)
    DVE: batched reciprocal -> rstd; srcm = src * mask (bf16);
         fused scalar_tensor_tensor: out = (h * rstd) + srcm
    DMA: grouped HWDGE loads/stores sized to amortize per-DMA overhead

Sharding: batch dim 32 -> 4 batches per core x 8 cores (data parallel,
weights replicated).
"""

import numpy as np
from ml_dtypes import bfloat16

import concourse.bass as bass
import concourse.tile as tile
from concourse import bacc, mybir
from concourse.bass_utils import run_bass_kernel_spmd

B, E, C, D, NA, MP = 32, 256, 7, 256, 8, 9
NCORES = 8
BC = B // NCORES                 # batches per core
ROWS = BC * E * C                # 7168 output rows per core
NTILES = ROWS // 128             # 56
K = 120                          # lhsT contraction dim
LN_EPS = 1e-5

_kernel_cache = {}


def _build_bass():
    nc = bacc.Bacc(None, target_bir_lowering=False, debug=False)
    f32 = mybir.dt.float32
    bf16 = mybir.dt.bfloat16

    lhsT = nc.declare_dram_parameter("lhsT", [K, ROWS], bf16, isOutput=False)
    w_hl = nc.declare_dram_parameter("w_hl", [K, 2 * D], bf16, isOutput=False)
    src_ext = nc.declare_dram_parameter("src_ext", [134, D], bf16, isOutput=False)
    m_cols = nc.declare_dram_parameter("m_cols", [128, NTILES], f32, isOutput=False)

    out = nc.declare_dram_parameter("out", [ROWS, D], f32, isOutput=True)

    G = 7                                    # tiles per group (DMA + stats batch)
    NG = NTILES // G                         # 8 groups
    with tile.TileContext(nc) as tc:
        with (
            tc.tile_pool(name="consts", bufs=1) as consts,
            tc.tile_pool(name="lhs", bufs=4) as lhs_pool,
            tc.tile_pool(name="psum", bufs=8, space="PSUM") as psum_pool,
            tc.tile_pool(name="stats", bufs=4) as stats_pool,
            tc.tile_pool(name="outs", bufs=3) as out_pool,
            tc.tile_pool(name="srcm_p", bufs=14) as srcm_pool,
            tc.tile_pool(name="scratch", bufs=1) as scratch_pool,
        ):
            # w_hl + the (tiny) first lhs tile first: they gate the first
            # matmul; src/mcol follow (they gate DVE's srcm prefetch work)
            SIZES = [1, 2, 4, 7, 7, 7, 7, 7, 7, 4, 2, 1]
            OFFS = [sum(SIZES[:i]) for i in range(len(SIZES))]
            NGV = len(SIZES)
            whl_sb = consts.tile([K, 2 * D], bf16, tag="whl")
            nc.sync.dma_start(out=whl_sb, in_=w_hl[:, :])
            lhs_tiles = {}
            lhs_sb = lhs_pool.tile([K, SIZES[0] * 128], bf16, tag="lhs")
            nc.sync.dma_start(out=lhs_sb, in_=lhsT[:, 0:SIZES[0] * 128])
            lhs_tiles[0] = lhs_sb
            # source-row variants, packed so block w starts at src_ext row w.
            # Tile t needs rows (128t+p) % 7, i.e. block w = (2t) % 7.
            src_sb = consts.tile([128, 7, D], bf16, tag="src")
            src_base = src_ext[:, :]
            nc.sync.dma_start(
                out=src_sb,
                in_=bass.AP(
                    tensor=src_base.tensor, offset=src_base.offset,
                    ap=[[D, 128], [D, 7], [1, D]],
                ),
            )
            mcol_sb = consts.tile([128, NTILES], f32, tag="mcol")
            nc.sync.dma_start(out=mcol_sb, in_=m_cols[:, :])
            eps_sb = consts.tile([128, 1], f32, tag="eps")
            nc.vector.memset(eps_sb, LN_EPS)
            for gp in (1, 2):
                t0, sz = OFFS[gp], SIZES[gp]
                lhs_sb = lhs_pool.tile([K, sz * 128], bf16, tag="lhs")
                nc.sync.dma_start(
                    out=lhs_sb, in_=lhsT[:, t0 * 128:(t0 + sz) * 128]
                )
                lhs_tiles[gp] = lhs_sb
            sq_garbage = scratch_pool.tile([128, D], bf16, tag="sqg")

            for g in range(NGV):
                t0, sz = OFFS[g], SIZES[g]
                lhs_sb = lhs_tiles.pop(g)
                ssq = stats_pool.tile([128, sz], f32, tag="ssq")
                rstd = stats_pool.tile([128, sz], f32, tag="rstd")
                out_g = out_pool.tile([128, sz, D], f32, tag="out")
                # srcm tiles depend only on constants — emit first so the
                # vector engine can run them while PE/ACT fill the group
                srcms = []
                for j in range(sz):
                    t = t0 + j
                    srcm = srcm_pool.tile([128, D], bf16, tag="srcm")
                    nc.vector.tensor_scalar_mul(
                        out=srcm, in0=src_sb[:, (2 * t) % 7, :],
                        scalar1=mcol_sb[:, t:t + 1],
                    )
                    srcms.append(srcm)
                h_tiles = []
                for j in range(sz):
                    h_ps = psum_pool.tile([128, D], f32, tag="h")
                    h_tiles.append(h_ps)
                nb1 = min(3, sz)          # first sqrt/recip batch size
                for j in range(sz):
                    lhsT_t = lhs_sb[:, j * 128:(j + 1) * 128]
                    h_ps = h_tiles[j]
                    nc.tensor.matmul(h_ps, lhsT_t, whl_sb[:, 0:D],
                                     start=True, stop=False)
                    nc.tensor.matmul(h_ps, lhsT_t, whl_sb[:, D:2 * D],
                                     start=False, stop=True)
                    # sum of squares per row (mean is 0 by weight centering);
                    # the last tile of big groups goes through DVE bn_stats
                    # to offload the ACT engine
                    if j < sz - 1 or sz < 7:
                        nc.scalar.activation(
                            out=sq_garbage, in_=h_ps,
                            func=mybir.ActivationFunctionType.Square,
                            accum_out=ssq[:, j:j + 1],
                        )
                    else:
                        st6 = stats_pool.tile([128, 6], f32, tag="st6")
                        nc.vector.bn_stats(out=st6, in_=h_ps)
                        mv = stats_pool.tile([128, 2], f32, tag="mv")
                        nc.vector.bn_aggr(out=mv, in_=st6)
                        nc.vector.tensor_scalar_mul(
                            out=ssq[:, j:j + 1], in0=mv[:, 1:2],
                            scalar1=float(D),
                        )
                    if j == nb1 - 1:
                        # early rstd for the first batch: shortens the
                        # stats -> STT dependency chain
                        nc.scalar.activation(
                            out=rstd[:, 0:nb1], in_=ssq[:, 0:nb1],
                            func=mybir.ActivationFunctionType.Sqrt,
                            bias=eps_sb[:, :], scale=1.0 / D,
                        )
                        nc.vector.reciprocal(out=rstd[:, 0:nb1],
                                             in_=rstd[:, 0:nb1])
                if sz > nb1:
                    nc.scalar.activation(
                        out=rstd[:, nb1:sz], in_=ssq[:, nb1:sz],
                        func=mybir.ActivationFunctionType.Sqrt,
                        bias=eps_sb[:, :], scale=1.0 / D,
                    )
                    nc.vector.reciprocal(out=rstd[:, nb1:sz],
                                         in_=rstd[:, nb1:sz])
                for j in range(sz):
                    nc.vector.scalar_tensor_tensor(
                        out=out_g[:, j, :], in0=h_tiles[j],
                        scalar=rstd[:, j:j + 1], in1=srcms[j],
                        op0=mybir.AluOpType.mult, op1=mybir.AluOpType.add,
                    )
                base = t0 * 128
                nc.sync.dma_start(
                    out=out[base:base + nb1 * 128, :].rearrange(
                        "(j p) d -> p j d", j=nb1),
                    in_=out_g[:, 0:nb1, :],
                )
                if sz > nb1:
                    nc.sync.dma_start(
                        out=out[base + nb1 * 128:base + sz * 128, :].rearrange(
                            "(j p) d -> p j d", j=sz - nb1),
                        in_=out_g[:, nb1:sz, :],
                    )
                if g + 3 < NGV:
                    tn, szn = OFFS[g + 3], SIZES[g + 3]
                    lhs_nx = lhs_pool.tile([K, szn * 128], bf16, tag="lhs")
                    nc.sync.dma_start(
                        out=lhs_nx, in_=lhsT[:, tn * 128:(tn + szn) * 128],
                    )
                    lhs_tiles[g + 3] = lhs_nx
    nc.finalize()
    return nc


def _prepare(inputs):
    """Host-side weight folding + per-core input shards."""
    f32 = np.float32
    W = np.asarray(inputs["W_combine"], f32)
    Wc = W[0:256]
    Wx = [W[256 * (i + 1):256 * (i + 2)] for i in range(6)]

    R = np.zeros((101, D), f32)
    R[0:53] = np.asarray(inputs["card_table"], f32) @ Wc
    R[53:62] = np.asarray(inputs["hero_table"], f32) @ Wx[0]
    R[62:71] = np.asarray(inputs["acting_table"], f32) @ Wx[1]
    R[71:81] = np.asarray(inputs["nump_table"], f32) @ Wx[2]
    R[81] = (np.asarray(inputs["b_scalar"], f32) @ Wx[3]
             + np.asarray(inputs["b_bet"], f32) @ Wx[4]
             + np.asarray(inputs["b_action"], f32) @ Wx[5]
             + np.asarray(inputs["b_combine"], f32))
    R[82:84] = np.asarray(inputs["W_scalar"], f32) @ Wx[3]
    R[84:93] = np.asarray(inputs["W_bet"], f32) @ Wx[4]
    R[93:101] = np.asarray(inputs["W_action"], f32) @ Wx[5]
    R = R - R.mean(axis=1, keepdims=True)

    rhs_full = np.vstack([R, R[82:101]])            # (120, 256)
    w_hi = rhs_full.astype(bfloat16)
    w_lo = (rhs_full - w_hi.astype(f32)).astype(bfloat16)
    w_hl = np.concatenate([w_hi, w_lo], axis=1)     # (120, 512)

    # source rows extended periodically so every tile variant is a slice
    src_ids = np.array([0, 0, 0, 0, 0, 1, 1])
    src7 = np.asarray(inputs["source_table"], f32)[src_ids]      # (7, 256)
    src_ext = src7[np.arange(134) % 7].astype(bfloat16)          # (134, 256)

    emask = (np.arange(E)[None, :]
             < np.asarray(inputs["seq_lengths"])[:, None]).astype(f32)  # (B, E)

    card = np.asarray(inputs["card_ids"]).reshape(B, E * C)
    hero = np.repeat(np.asarray(inputs["hero_pos"]), C, axis=1)  # (B, E*C)
    act = np.repeat(np.asarray(inputs["acting_pos"]), C, axis=1)
    nump = np.repeat(np.asarray(inputs["num_players"]), C, axis=1)
    feats = np.concatenate(
        [np.asarray(inputs["scalars"], f32),
         np.asarray(inputs["bets"], f32),
         np.asarray(inputs["action"], f32)], axis=-1)            # (B, E, 19)
    f_hi = feats.astype(bfloat16).astype(f32)
    f_lo = (feats - f_hi)

    in_maps = []
    for m in range(NCORES):
        bs = slice(m * BC, (m + 1) * BC)
        n = ROWS
        cols = np.arange(n)
        lhsT = np.zeros((K, n), f32)
        lhsT[card[bs].reshape(-1), cols] = 1.0
        lhsT[53 + hero[bs].reshape(-1), cols] = 1.0
        lhsT[62 + act[bs].reshape(-1), cols] = 1.0
        lhsT[71 + nump[bs].reshape(-1), cols] = 1.0
        lhsT[81, :] = 1.0
        lhsT[82:101, :] = np.repeat(
            f_hi[bs].reshape(-1, 19), C, axis=0).T
        lhsT[101:120, :] = np.repeat(
            f_lo[bs].reshape(-1, 19), C, axis=0).T
        mrow = np.repeat(emask[bs].reshape(-1), C)               # (7168,)
        lhsT *= mrow[None, :]
        m_cols = mrow.reshape(NTILES, 128).T.copy()              # (128, 56)
        in_maps.append({
            "lhsT": lhsT.astype(bfloat16),
            "w_hl": w_hl,
            "src_ext": src_ext,
            "m_cols": m_cols.astype(f32),
        })
    mask_full = np.repeat(emask, C, axis=1).astype(f32)          # (B, E*C)
    return in_maps, mask_full


def kernel(**inputs):
    if "nc" not in _kernel_cache:
        _kernel_cache["nc"] = _build_bass()
    nc = _kernel_cache["nc"]
    in_maps, mask = _prepare(inputs)
    res = run_bass_kernel_spmd(nc, in_maps, list(range(NCORES)))
    embs = np.concatenate(
        [res.results[m]["out"].reshape(BC, E * C, D) for m in range(NCORES)],
        axis=0)
    return embs, mask


# revision 32
# speedup vs baseline: 1.0312x; 1.0133x over previous
"""EventSequenceEmbedder Trainium2 kernel (8-core data-parallel).

Strategy
--------
The reference computes, per (batch, event, card):
    h = concat(card_emb, hero_emb, acting_emb, nump_emb, scalars@Ws,
               bets@Wb, action@Wa) @ W_combine + b_combine
    out = LayerNorm(h) + source_emb,  zeroed for padded events.

Everything feeding h is linear, so the whole pre-LN graph folds into ONE
small matmul per output row:
    h[r, :] = A[r, :] @ W_tilde            A[r] in R^120
where A[r] packs one-hot card/hero/acting/nump ids, a bias-1, and the 19
raw float features (split into bf16 hi+lo rows for fp32-level accuracy),
and W_tilde rows are the host-projected tables (card_table@Wc, etc.).
W_tilde rows are mean-centered on the host so mean(h) == 0 analytically,
and W_tilde is split hi/lo into two bf16 matmuls accumulating in fp32
PSUM.  Masked (padded) events get all-zero A columns -> h == 0 -> LN
output 0.

Per 128-row tile on device (tiles processed in groups of ~7 that share
one lhs-load DMA, one store DMA pair, and batched sqrt/reciprocal):
    PE:  2 bf16 matmuls (W_hi half, W_lo half) -> h in PSUM
    ACT: Square with accum_out -> sum(h^2) per row (one tile per big
         group uses DVE bn_stats instead, to balance engine load);
         batched Sqrt(ssq/256 + eps)
    DVE: batched reciprocal -> rstd; srcm = src * mask (bf16);
         fused scalar_tensor_tensor: out = (h * rstd) + srcm
    DMA: grouped HWDGE loads/stores sized to amortize per-DMA overhead

Sharding: batch dim 32 -> 4 batches per core x 8 cores (data parallel,
weights replicated).
"""

import numpy as np
from ml_dtypes import bfloat16

import concourse.bass as bass
import concourse.tile as tile
from concourse import bacc, mybir
from concourse.bass_utils import run_bass_kernel_spmd

B, E, C, D, NA, MP = 32, 256, 7, 256, 8, 9
NCORES = 8
BC = B // NCORES                 # batches per core
ROWS = BC * E * C                # 7168 output rows per core
NTILES = ROWS // 128             # 56
K = 120                          # lhsT contraction dim
LN_EPS = 1e-5

_kernel_cache = {}


def _build_bass():
    nc = bacc.Bacc(None, target_bir_lowering=False, debug=False)
    f32 = mybir.dt.float32
    bf16 = mybir.dt.bfloat16

    lhsT = nc.declare_dram_parameter("lhsT", [K, ROWS], bf16, isOutput=False)
    w_hl = nc.declare_dram_parameter("w_hl", [K, 2 * D], bf16, isOutput=False)
    src_ext = nc.declare_dram_parameter("src_ext", [134, D], bf16, isOutput=False)
    m_cols = nc.declare_dram_parameter("m_cols", [128, NTILES], f32, isOutput=False)

    out = nc.declare_dram_parameter("out", [ROWS, D], f32, isOutput=True)

    G = 7                                    # tiles per group (DMA + stats batch)
    NG = NTILES // G                         # 8 groups
    with tile.TileContext(nc) as tc:
        with (
            tc.tile_pool(name="consts", bufs=1) as consts,
            tc.tile_pool(name="lhs", bufs=4) as lhs_pool,
            tc.tile_pool(name="psum", bufs=8, space="PSUM") as psum_pool,
            tc.tile_pool(name="stats", bufs=4) as stats_pool,
            tc.tile_pool(name="outs", bufs=3) as out_pool,
            tc.tile_pool(name="srcm_p", bufs=14) as srcm_pool,
            tc.tile_pool(name="scratch", bufs=1) as scratch_pool,
        ):
            # w_hl + the (tiny) first lhs tile first: they gate the first
            # matmul; src/mcol follow (they gate DVE's srcm prefetch work)
            SIZES = [1, 2, 4, 7, 7, 7, 7, 7, 7, 4, 2, 1]
            OFFS = [sum(SIZES[:i]) for i in range(len(SIZES))]
            NGV = len(SIZES)
            whl_sb = consts.tile([K, 2 * D], bf16, tag="whl")
            nc.sync.dma_start(out=whl_sb, in_=w_hl[:, :])
            lhs_tiles = {}
            lhs_sb = lhs_pool.tile([K, SIZES[0] * 128], bf16, tag="lhs")
            nc.sync.dma_start(out=lhs_sb, in_=lhsT[:, 0:SIZES[0] * 128])
            lhs_tiles[0] = lhs_sb
            # source-row variants, packed so block w starts at src_ext row w.
            # Tile t needs rows (128t+p) % 7, i.e. block w = (2t) % 7.
            src_sb = consts.tile([128, 7, D], bf16, tag="src")
            src_base = src_ext[:, :]
            nc.sync.dma_start(
                out=src_sb,
                in_=bass.AP(
                    tensor=src_base.tensor, offset=src_base.offset,
                    ap=[[D, 128], [D, 7], [1, D]],
                ),
            )
            mcol_sb = consts.tile([128, NTILES], f32, tag="mcol")
            nc.sync.dma_start(out=mcol_sb, in_=m_cols[:, :])
            eps_sb = consts.tile([128, 1], f32, tag="eps")
            nc.vector.memset(eps_sb, LN_EPS)
            for gp in (1, 2):
                t0, sz = OFFS[gp], SIZES[gp]
                lhs_sb = lhs_pool.tile([K, sz * 128], bf16, tag="lhs")
                nc.sync.dma_start(
                    out=lhs_sb, in_=lhsT[:, t0 * 128:(t0 + sz) * 128]
                )
                lhs_tiles[gp] = lhs_sb
            sq_garbage = scratch_pool.tile([128, D], bf16, tag="sqg")

            for g in range(NGV):
                t0, sz = OFFS[g], SIZES[g]
                lhs_sb = lhs_tiles.pop(g)
                ssq = stats_pool.tile([128, sz], f32, tag="ssq")
                rstd = stats_pool.tile([128, sz], f32, tag="rstd")
                out_g = out_pool.tile([128, sz, D], f32, tag="out")
                # srcm tiles depend only on constants — emit first so the
                # vector engine can run them while PE/ACT fill the group
                srcms = []
                for j in range(sz):
                    t = t0 + j
                    srcm = srcm_pool.tile([128, D], bf16, tag="srcm")
                    if j % 3 == 0:
                        mcol = mcol_sb[:, t:t + 1]
                        m_bcast = bass.AP(
                            tensor=mcol.tensor, offset=mcol.offset,
                            ap=[mcol.ap[0], [0, D]],
                        )
                        nc.gpsimd.tensor_tensor(
                            out=srcm, in0=src_sb[:, (2 * t) % 7, :],
                            in1=m_bcast, op=mybir.AluOpType.mult,
                        )
                    else:
                        nc.vector.tensor_scalar_mul(
                            out=srcm, in0=src_sb[:, (2 * t) % 7, :],
                            scalar1=mcol_sb[:, t:t + 1],
                        )
                    srcms.append(srcm)
                h_tiles = []
                for j in range(sz):
                    h_ps = psum_pool.tile([128, D], f32, tag="h")
                    h_tiles.append(h_ps)
                nb1 = min(3, sz)          # first sqrt/recip batch size
                for j in range(sz):
                    lhsT_t = lhs_sb[:, j * 128:(j + 1) * 128]
                    h_ps = h_tiles[j]
                    nc.tensor.matmul(h_ps, lhsT_t, whl_sb[:, 0:D],
                                     start=True, stop=False)
                    nc.tensor.matmul(h_ps, lhsT_t, whl_sb[:, D:2 * D],
                                     start=False, stop=True)
                    # sum of squares per row (mean is 0 by weight centering);
                    # the last tile of big groups goes through DVE bn_stats
                    # to offload the ACT engine
                    if j < sz - 1 or sz < 7:
                        nc.scalar.activation(
                            out=sq_garbage, in_=h_ps,
                            func=mybir.ActivationFunctionType.Square,
                            accum_out=ssq[:, j:j + 1],
                        )
                    else:
                        st6 = stats_pool.tile([128, 6], f32, tag="st6")
                        nc.vector.bn_stats(out=st6, in_=h_ps)
                        mv = stats_pool.tile([128, 2], f32, tag="mv")
                        nc.vector.bn_aggr(out=mv, in_=st6)
                        nc.vector.tensor_scalar_mul(
                            out=ssq[:, j:j + 1], in0=mv[:, 1:2],
                            scalar1=float(D),
                        )
                    if j == nb1 - 1:
                        # early rstd for the first batch: shortens the
                        # stats -> STT dependency chain
                        nc.scalar.activation(
                            out=rstd[:, 0:nb1], in_=ssq[:, 0:nb1],
                            func=mybir.ActivationFunctionType.Sqrt,
                            bias=eps_sb[:, :], scale=1.0 / D,
                        )
                        nc.vector.reciprocal(out=rstd[:, 0:nb1],
                                             in_=rstd[:, 0:nb1])
                if sz > nb1:
                    nc.scalar.activation(
                        out=rstd[:, nb1:sz], in_=ssq[:, nb1:sz],
                        func=mybir.ActivationFunctionType.Sqrt,
                        bias=eps_sb[:, :], scale=1.0 / D,
                    )
                    nc.vector.reciprocal(out=rstd[:, nb1:sz],
                                         in_=rstd[:, nb1:sz])
                for j in range(sz):
                    nc.vector.scalar_tensor_tensor(
                        out=out_g[:, j, :], in0=h_tiles[j],
                        scalar=rstd[:, j:j + 1], in1=srcms[j],
                        op0=mybir.AluOpType.mult, op1=mybir.AluOpType.add,
                    )
                base = t0 * 128
                nc.sync.dma_start(
                    out=out[base:base + nb1 * 128, :].rearrange(
                        "(j p) d -> p j d", j=nb1),
                    in_=out_g[:, 0:nb1, :],
                )
                if sz > nb1:
                    nc.sync.dma_start(
                        out=out[base + nb1 * 128:base + sz * 128, :].rearrange(
                            "(j p) d -> p j d", j=sz - nb1),
                        in_=out_g[:, nb1:sz, :],
                    )
                if g + 3 < NGV:
                    tn, szn = OFFS[g + 3], SIZES[g + 3]
                    lhs_nx = lhs_pool.tile([K, szn * 128], bf16, tag="lhs")
                    nc.sync.dma_start(
                        out=lhs_nx, in_=lhsT[:, tn * 128:(tn + szn) * 128],
                    )
                    lhs_tiles[g + 3] = lhs_nx
    nc.finalize()
    return nc


def _prepare(inputs):
    """Host-side weight folding + per-core input shards."""
    f32 = np.float32
    W = np.asarray(inputs["W_combine"], f32)
    Wc = W[0:256]
    Wx = [W[256 * (i + 1):256 * (i + 2)] for i in range(6)]

    R = np.zeros((101, D), f32)
    R[0:53] = np.asarray(inputs["card_table"], f32) @ Wc
    R[53:62] = np.asarray(inputs["hero_table"], f32) @ Wx[0]
    R[62:71] = np.asarray(inputs["acting_table"], f32) @ Wx[1]
    R[71:81] = np.asarray(inputs["nump_table"], f32) @ Wx[2]
    R[81] = (np.asarray(inputs["b_scalar"], f32) @ Wx[3]
             + np.asarray(inputs["b_bet"], f32) @ Wx[4]
             + np.asarray(inputs["b_action"], f32) @ Wx[5]
             + np.asarray(inputs["b_combine"], f32))
    R[82:84] = np.asarray(inputs["W_scalar"], f32) @ Wx[3]
    R[84:93] = np.asarray(inputs["W_bet"], f32) @ Wx[4]
    R[93:101] = np.asarray(inputs["W_action"], f32) @ Wx[5]
    R = R - R.mean(axis=1, keepdims=True)

    rhs_full = np.vstack([R, R[82:101]])            # (120, 256)
    w_hi = rhs_full.astype(bfloat16)
    w_lo = (rhs_full - w_hi.astype(f32)).astype(bfloat16)
    w_hl = np.concatenate([w_hi, w_lo], axis=1)     # (120, 512)

    # source rows extended periodically so every tile variant is a slice
    src_ids = np.array([0, 0, 0, 0, 0, 1, 1])
    src7 = np.asarray(inputs["source_table"], f32)[src_ids]      # (7, 256)
    src_ext = src7[np.arange(134) % 7].astype(bfloat16)          # (134, 256)

    emask = (np.arange(E)[None, :]
             < np.asarray(inputs["seq_lengths"])[:, None]).astype(f32)  # (B, E)

    card = np.asarray(inputs["card_ids"]).reshape(B, E * C)
    hero = np.repeat(np.asarray(inputs["hero_pos"]), C, axis=1)  # (B, E*C)
    act = np.repeat(np.asarray(inputs["acting_pos"]), C, axis=1)
    nump = np.repeat(np.asarray(inputs["num_players"]), C, axis=1)
    feats = np.concatenate(
        [np.asarray(inputs["scalars"], f32),
         np.asarray(inputs["bets"], f32),
         np.asarray(inputs["action"], f32)], axis=-1)            # (B, E, 19)
    f_hi = feats.astype(bfloat16).astype(f32)
    f_lo = (feats - f_hi)

    in_maps = []
    for m in range(NCORES):
        bs = slice(m * BC, (m + 1) * BC)
        n = ROWS
        cols = np.arange(n)
        lhsT = np.zeros((K, n), f32)
        lhsT[card[bs].reshape(-1), cols] = 1.0
        lhsT[53 + hero[bs].reshape(-1), cols] = 1.0
        lhsT[62 + act[bs].reshape(-1), cols] = 1.0
        lhsT[71 + nump[bs].reshape(-1), cols] = 1.0
        lhsT[81, :] = 1.0
        lhsT[82:101, :] = np.repeat(
            f_hi[bs].reshape(-1, 19), C, axis=0).T
        lhsT[101:120, :] = np.repeat(
            f_lo[bs].reshape(-1, 19), C, axis=0).T
        mrow = np.repeat(emask[bs].reshape(-1), C)               # (7168,)
        lhsT *= mrow[None, :]
        m_cols = mrow.reshape(NTILES, 128).T.copy()              # (128, 56)
        in_maps.append({
            "lhsT": lhsT.astype(bfloat16),
            "w_hl": w_hl,
            "src_ext": src_ext,
            "m_cols": m_cols.astype(f32),
        })
    mask_full = np.repeat(emask, C, axis=1).astype(f32)          # (B, E*C)
    return in_maps, mask_full


def kernel(**inputs):
    if "nc" not in _kernel_cache:
        _kernel_cache["nc"] = _build_bass()
    nc = _kernel_cache["nc"]
    in_maps, mask = _prepare(inputs)
    res = run_bass_kernel_spmd(nc, in_maps, list(range(NCORES)))
    embs = np.concatenate(
        [res.results[m]["out"].reshape(BC, E * C, D) for m in range(NCORES)],
        axis=0)
    return embs, mask


# revision 33
# speedup vs baseline: 1.0334x; 1.0022x over previous
"""EventSequenceEmbedder Trainium2 kernel (8-core data-parallel).

Strategy
--------
The reference computes, per (batch, event, card):
    h = concat(card_emb, hero_emb, acting_emb, nump_emb, scalars@Ws,
               bets@Wb, action@Wa) @ W_combine + b_combine
    out = LayerNorm(h) + source_emb,  zeroed for padded events.

Everything feeding h is linear, so the whole pre-LN graph folds into ONE
small matmul per output row:
    h[r, :] = A[r, :] @ W_tilde            A[r] in R^120
where A[r] packs one-hot card/hero/acting/nump ids, a bias-1, and the 19
raw float features (split into bf16 hi+lo rows for fp32-level accuracy),
and W_tilde rows are the host-projected tables (card_table@Wc, etc.).
W_tilde rows are mean-centered on the host so mean(h) == 0 analytically,
and W_tilde is split hi/lo into two bf16 matmuls accumulating in fp32
PSUM.  Masked (padded) events get all-zero A columns -> h == 0 -> LN
output 0.

Per 128-row tile on device (tiles processed in groups of ~7 that share
one lhs-load DMA, one store DMA pair, and batched sqrt/reciprocal):
    PE:  2 bf16 matmuls (W_hi half, W_lo half) -> h in PSUM
    ACT: Square with accum_out -> sum(h^2) per row (one tile per big
         group uses DVE bn_stats instead, to balance engine load);
         batched Sqrt(ssq/256 + eps)
    DVE: batched reciprocal -> rstd; srcm = src * mask (bf16);
         fused scalar_tensor_tensor: out = (h * rstd) + srcm
    DMA: grouped HWDGE loads/stores sized to amortize per-DMA overhead

Sharding: batch dim 32 -> 4 batches per core x 8 cores (data parallel,
weights replicated).
"""

import numpy as np
from ml_dtypes import bfloat16

import concourse.bass as bass
import concourse.tile as tile
from concourse import bacc, mybir
from concourse.bass_utils import run_bass_kernel_spmd

B, E, C, D, NA, MP = 32, 256, 7, 256, 8, 9
NCORES = 8
BC = B // NCORES                 # batches per core
ROWS = BC * E * C                # 7168 output rows per core
NTILES = ROWS // 128             # 56
K = 120                          # lhsT contraction dim
LN_EPS = 1e-5

_kernel_cache = {}


def _build_bass():
    nc = bacc.Bacc(None, target_bir_lowering=False, debug=False)
    f32 = mybir.dt.float32
    bf16 = mybir.dt.bfloat16

    lhsT = nc.declare_dram_parameter("lhsT", [K, ROWS], bf16, isOutput=False)
    w_hl = nc.declare_dram_parameter("w_hl", [K, 2 * D], bf16, isOutput=False)
    src_ext = nc.declare_dram_parameter("src_ext", [134, D], bf16, isOutput=False)
    m_cols = nc.declare_dram_parameter("m_cols", [128, NTILES], f32, isOutput=False)

    out = nc.declare_dram_parameter("out", [ROWS, D], f32, isOutput=True)

    G = 7                                    # tiles per group (DMA + stats batch)
    NG = NTILES // G                         # 8 groups
    with tile.TileContext(nc) as tc:
        with (
            tc.tile_pool(name="consts", bufs=1) as consts,
            tc.tile_pool(name="lhs", bufs=4) as lhs_pool,
            tc.tile_pool(name="psum", bufs=8, space="PSUM") as psum_pool,
            tc.tile_pool(name="stats", bufs=4) as stats_pool,
            tc.tile_pool(name="outs", bufs=3) as out_pool,
            tc.tile_pool(name="srcm_p", bufs=14) as srcm_pool,
            tc.tile_pool(name="scratch", bufs=1) as scratch_pool,
        ):
            # w_hl + the (tiny) first lhs tile first: they gate the first
            # matmul; src/mcol follow (they gate DVE's srcm prefetch work)
            SIZES = [1, 2, 4, 7, 7, 7, 7, 7, 7, 4, 2, 1]
            OFFS = [sum(SIZES[:i]) for i in range(len(SIZES))]
            NGV = len(SIZES)
            whl_sb = consts.tile([K, 2 * D], bf16, tag="whl")
            nc.sync.dma_start(out=whl_sb, in_=w_hl[:, :])
            lhs_tiles = {}
            lhs_sb = lhs_pool.tile([K, SIZES[0] * 128], bf16, tag="lhs")
            nc.sync.dma_start(out=lhs_sb, in_=lhsT[:, 0:SIZES[0] * 128])
            lhs_tiles[0] = lhs_sb
            # source-row variants, packed so block w starts at src_ext row w.
            # Tile t needs rows (128t+p) % 7, i.e. block w = (2t) % 7.
            src_sb = consts.tile([128, 7, D], bf16, tag="src")
            src_base = src_ext[:, :]
            nc.sync.dma_start(
                out=src_sb,
                in_=bass.AP(
                    tensor=src_base.tensor, offset=src_base.offset,
                    ap=[[D, 128], [D, 7], [1, D]],
                ),
            )
            mcol_sb = consts.tile([128, NTILES], f32, tag="mcol")
            nc.sync.dma_start(out=mcol_sb, in_=m_cols[:, :])
            eps_sb = consts.tile([128, 1], f32, tag="eps")
            nc.vector.memset(eps_sb, LN_EPS)
            for gp in (1, 2):
                t0, sz = OFFS[gp], SIZES[gp]
                lhs_sb = lhs_pool.tile([K, sz * 128], bf16, tag="lhs")
                nc.sync.dma_start(
                    out=lhs_sb, in_=lhsT[:, t0 * 128:(t0 + sz) * 128]
                )
                lhs_tiles[gp] = lhs_sb
            sq_garbage = scratch_pool.tile([128, D], bf16, tag="sqg")

            for g in range(NGV):
                t0, sz = OFFS[g], SIZES[g]
                lhs_sb = lhs_tiles.pop(g)
                ssq = stats_pool.tile([128, sz], f32, tag="ssq")
                rstd = stats_pool.tile([128, sz], f32, tag="rstd")
                out_g = out_pool.tile([128, sz, D], f32, tag="out")
                # srcm tiles depend only on constants — emit first so the
                # vector engine can run them while PE/ACT fill the group
                srcms = []
                for j in range(sz):
                    t = t0 + j
                    srcm = srcm_pool.tile([128, D], bf16, tag="srcm")
                    if j in (1, 4):
                        mcol = mcol_sb[:, t:t + 1]
                        m_bcast = bass.AP(
                            tensor=mcol.tensor, offset=mcol.offset,
                            ap=[mcol.ap[0], [0, D]],
                        )
                        nc.gpsimd.tensor_tensor(
                            out=srcm, in0=src_sb[:, (2 * t) % 7, :],
                            in1=m_bcast, op=mybir.AluOpType.mult,
                        )
                    else:
                        nc.vector.tensor_scalar_mul(
                            out=srcm, in0=src_sb[:, (2 * t) % 7, :],
                            scalar1=mcol_sb[:, t:t + 1],
                        )
                    srcms.append(srcm)
                h_tiles = []
                for j in range(sz):
                    h_ps = psum_pool.tile([128, D], f32, tag="h")
                    h_tiles.append(h_ps)
                nb1 = min(3, sz)          # first sqrt/recip batch size
                for j in range(sz):
                    lhsT_t = lhs_sb[:, j * 128:(j + 1) * 128]
                    h_ps = h_tiles[j]
                    nc.tensor.matmul(h_ps, lhsT_t, whl_sb[:, 0:D],
                                     start=True, stop=False)
                    nc.tensor.matmul(h_ps, lhsT_t, whl_sb[:, D:2 * D],
                                     start=False, stop=True)
                    # sum of squares per row (mean is 0 by weight centering);
                    # the last tile of big groups goes through DVE bn_stats
                    # to offload the ACT engine
                    if j < sz - 1 or sz < 7:
                        nc.scalar.activation(
                            out=sq_garbage, in_=h_ps,
                            func=mybir.ActivationFunctionType.Square,
                            accum_out=ssq[:, j:j + 1],
                        )
                    else:
                        st6 = stats_pool.tile([128, 6], f32, tag="st6")
                        nc.vector.bn_stats(out=st6, in_=h_ps)
                        mv = stats_pool.tile([128, 2], f32, tag="mv")
                        nc.vector.bn_aggr(out=mv, in_=st6)
                        nc.vector.tensor_scalar_mul(
                            out=ssq[:, j:j + 1], in0=mv[:, 1:2],
                            scalar1=float(D),
                        )
                    if j == nb1 - 1:
                        # early rstd for the first batch: shortens the
                        # stats -> STT dependency chain
                        nc.scalar.activation(
                            out=rstd[:, 0:nb1], in_=ssq[:, 0:nb1],
                            func=mybir.ActivationFunctionType.Sqrt,
                            bias=eps_sb[:, :], scale=1.0 / D,
                        )
                        nc.vector.reciprocal(out=rstd[:, 0:nb1],
                                             in_=rstd[:, 0:nb1])
                if sz > nb1:
                    nc.scalar.activation(
                        out=rstd[:, nb1:sz], in_=ssq[:, nb1:sz],
                        func=mybir.ActivationFunctionType.Sqrt,
                        bias=eps_sb[:, :], scale=1.0 / D,
                    )
                    nc.vector.reciprocal(out=rstd[:, nb1:sz],
                                         in_=rstd[:, nb1:sz])
                for j in range(sz):
                    nc.vector.scalar_tensor_tensor(
                        out=out_g[:, j, :], in0=h_tiles[j],
                        scalar=rstd[:, j:j + 1], in1=srcms[j],
                        op0=mybir.AluOpType.mult, op1=mybir.AluOpType.add,
                    )
                base = t0 * 128
                nc.sync.dma_start(
                    out=out[base:base + nb1 * 128, :].rearrange(
                        "(j p) d -> p j d", j=nb1),
                    in_=out_g[:, 0:nb1, :],
                )
                if sz > nb1:
                    nc.sync.dma_start(
                        out=out[base + nb1 * 128:base + sz * 128, :].rearrange(
                            "(j p) d -> p j d", j=sz - nb1),
                        in_=out_g[:, nb1:sz, :],
                    )
                if g + 3 < NGV:
                    tn, szn = OFFS[g + 3], SIZES[g + 3]
                    lhs_nx = lhs_pool.tile([K, szn * 128], bf16, tag="lhs")
                    nc.sync.dma_start(
                        out=lhs_nx, in_=lhsT[:, tn * 128:(tn + szn) * 128],
                    )
                    lhs_tiles[g + 3] = lhs_nx
    nc.finalize()
    return nc


def _prepare(inputs):
    """Host-side weight folding + per-core input shards."""
    f32 = np.float32
    W = np.asarray(inputs["W_combine"], f32)
    Wc = W[0:256]
    Wx = [W[256 * (i + 1):256 * (i + 2)] for i in range(6)]

    R = np.zeros((101, D), f32)
    R[0:53] = np.asarray(inputs["card_table"], f32) @ Wc
    R[53:62] = np.asarray(inputs["hero_table"], f32) @ Wx[0]
    R[62:71] = np.asarray(inputs["acting_table"], f32) @ Wx[1]
    R[71:81] = np.asarray(inputs["nump_table"], f32) @ Wx[2]
    R[81] = (np.asarray(inputs["b_scalar"], f32) @ Wx[3]
             + np.asarray(inputs["b_bet"], f32) @ Wx[4]
             + np.asarray(inputs["b_action"], f32) @ Wx[5]
             + np.asarray(inputs["b_combine"], f32))
    R[82:84] = np.asarray(inputs["W_scalar"], f32) @ Wx[3]
    R[84:93] = np.asarray(inputs["W_bet"], f32) @ Wx[4]
    R[93:101] = np.asarray(inputs["W_action"], f32) @ Wx[5]
    R = R - R.mean(axis=1, keepdims=True)

    rhs_full = np.vstack([R, R[82:101]])            # (120, 256)
    w_hi = rhs_full.astype(bfloat16)
    w_lo = (rhs_full - w_hi.astype(f32)).astype(bfloat16)
    w_hl = np.concatenate([w_hi, w_lo], axis=1)     # (120, 512)

    # source rows extended periodically so every tile variant is a slice
    src_ids = np.array([0, 0, 0, 0, 0, 1, 1])
    src7 = np.asarray(inputs["source_table"], f32)[src_ids]      # (7, 256)
    src_ext = src7[np.arange(134) % 7].astype(bfloat16)          # (134, 256)

    emask = (np.arange(E)[None, :]
             < np.asarray(inputs["seq_lengths"])[:, None]).astype(f32)  # (B, E)

    card = np.asarray(inputs["card_ids"]).reshape(B, E * C)
    hero = np.repeat(np.asarray(inputs["hero_pos"]), C, axis=1)  # (B, E*C)
    act = np.repeat(np.asarray(inputs["acting_pos"]), C, axis=1)
    nump = np.repeat(np.asarray(inputs["num_players"]), C, axis=1)
    feats = np.concatenate(
        [np.asarray(inputs["scalars"], f32),
         np.asarray(inputs["bets"], f32),
         np.asarray(inputs["action"], f32)], axis=-1)            # (B, E, 19)
    f_hi = feats.astype(bfloat16).astype(f32)
    f_lo = (feats - f_hi)

    in_maps = []
    for m in range(NCORES):
        bs = slice(m * BC, (m + 1) * BC)
        n = ROWS
        cols = np.arange(n)
        lhsT = np.zeros((K, n), f32)
        lhsT[card[bs].reshape(-1), cols] = 1.0
        lhsT[53 + hero[bs].reshape(-1), cols] = 1.0
        lhsT[62 + act[bs].reshape(-1), cols] = 1.0
        lhsT[71 + nump[bs].reshape(-1), cols] = 1.0
        lhsT[81, :] = 1.0
        lhsT[82:101, :] = np.repeat(
            f_hi[bs].reshape(-1, 19), C, axis=0).T
        lhsT[101:120, :] = np.repeat(
            f_lo[bs].reshape(-1, 19), C, axis=0).T
        mrow = np.repeat(emask[bs].reshape(-1), C)               # (7168,)
        lhsT *= mrow[None, :]
        m_cols = mrow.reshape(NTILES, 128).T.copy()              # (128, 56)
        in_maps.append({
            "lhsT": lhsT.astype(bfloat16),
            "w_hl": w_hl,
            "src_ext": src_ext,
            "m_cols": m_cols.astype(f32),
        })
    mask_full = np.repeat(emask, C, axis=1).astype(f32)          # (B, E*C)
    return in_maps, mask_full


def kernel(**inputs):
    if "nc" not in _kernel_cache:
        _kernel_cache["nc"] = _build_bass()
    nc = _kernel_cache["nc"]
    in_maps, mask = _prepare(inputs)
    res = run_bass_kernel_spmd(nc, in_maps, list(range(NCORES)))
    embs = np.concatenate(
        [res.results[m]["out"].reshape(BC, E * C, D) for m in range(NCORES)],
        axis=0)
    return embs, mask
